# revision 30
# baseline (speedup 1.0000x reference)
"""Trainium2 Bass kernel for the DGCL GNN (3 GIN conv layers + 8-factor
disentangled head + global add pool).

Self-contained: host-side numpy preprocessing (graph partitioning /
weight packing / one-hot construction), an SPMD Bass/Tile device program
for 8 NeuronCores, and the gather/unshard glue.

Structure of the computation (mathematically identical to the reference):
  - The K=8 disentangled head factors share the same edge aggregation, and
    their per-factor MLPs concatenate into [128,128] dense / block-diagonal
    matmuls.  So the network is 5 uniform layers:
        z = h + scatter_add(gather(h, src), dst)
        v = relu(z @ W1 + b1) @ W2 + b2
        h' = BN(v) (+ relu for layers 0,1,3)
    followed by a per-graph add-pool.
  - Nodes (and their incoming edges) are sharded contiguously across the 8
    cores.  Edge gathers read a bf16 replica of h from local DRAM via
    dma_gather (int16 indices -> lo/hi half split); aggregation happens as
    bf16 one-hot matmuls accumulating in fp32 PSUM.  The scatter one-hots
    are precomputed on the host in fp8 (exact 0/1) and streamed from DRAM,
    so the Vector engine does no is_equal work.  The self term is added
    in fp32 from an SBUF-resident transposed copy of the core's own shard.
    The h replica is refreshed each layer with an AllGather; BN statistics
    and the pooled output use AllReduce.
"""

import math
from contextlib import ExitStack

import numpy as np

import concourse.bacc as bacc
import concourse.bass as bass
import concourse.mybir as mybir
import concourse.tile as tile
from concourse.bass_utils import run_bass_kernel_spmd
from concourse.masks import make_identity

P = 128
F = 128
GOUT = 512          # output graph rows (harness G = 512)
BN_EPS = 1e-5
RELU_AFTER = [True, True, False, True, False]
f32 = mybir.dt.float32
bf16 = mybir.dt.bfloat16
fp8 = mybir.dt.float8e4
i16 = mybir.dt.int16


class Cfg:
    def __init__(self, N, C, Th, group_blocks=3):
        self.N = N                      # real node count
        self.C = C                      # cores
        self.Nshard = -(-N // (C * P)) * P
        self.Np = self.Nshard * C
        self.B = self.Nshard // P       # dst blocks per core
        self.Th = Th                    # 128-edge tiles per block-half
        self.Gb = group_blocks          # blocks per gather-call group
        self.n_groups = -(-self.B // group_blocks)
        self.H = self.Np // 2
        self.total_tiles = self.B * 2 * Th
        self.tile_base = None           # filled by fill_groups
        self.nb_of_group = None

    def fill_groups(self):
        tb, bases, nbs = 0, [], []
        for g in range(self.n_groups):
            nb = min((g + 1) * self.Gb, self.B) - g * self.Gb
            bases.append(tb)
            nbs.append(nb)
            tb += 2 * nb * self.Th
        self.tile_base, self.nb_of_group = bases, nbs
        return self


# ----------------------------------------------------------------------------
# Host-side preprocessing
# ----------------------------------------------------------------------------

def _blockdiag(W):
    K_, d_, _ = W.shape
    out = np.zeros((K_ * d_, K_ * d_), np.float32)
    for k in range(K_):
        out[k * d_:(k + 1) * d_, k * d_:(k + 1) * d_] = W[k]
    return out


def prep_host(x, edge_index, batch, params, C=8, group_blocks=3):
    """Returns (cfg, in_maps) ready for the device program."""
    x = np.asarray(x, np.float32)
    N = x.shape[0]
    src = np.asarray(edge_index[0], np.int64)
    dst = np.asarray(edge_index[1], np.int64)
    batch = np.asarray(batch, np.int64)

    order = np.argsort(dst, kind="stable")
    s_sorted = src[order]
    d_sorted = dst[order]

    gblock = d_sorted // P                      # global dst block id
    slot = d_sorted % P

    NshardP = -(-N // (C * P)) * P
    B = NshardP // P
    Np = NshardP * C
    H = Np // 2
    assert H <= 32767 and Np - H <= 32767, "half-table exceeds int16 range"

    # split each block's edge list by source half (int16 index limit)
    is_hi = (s_sorted >= H).astype(np.int64)
    halfkey = gblock * 2 + is_hi
    counts2 = np.bincount(halfkey, minlength=C * B * 2)
    Th = max(1, int(math.ceil(counts2.max() / P)))
    cfg = Cfg(N, C, Th, group_blocks).fill_groups()
    Gb = group_blocks

    total_tiles = cfg.total_tiles
    idx16 = np.zeros((C, 16, total_tiles * 8), np.int16)
    # fp8 one-hot tiles, one even and one odd plane per tile (paired 512B
    # gathers): oh[c, e_p, tile*256 + (src&1)*128 + slot(e)] = 1
    oh = np.zeros((C, P, total_tiles * 2 * P), mybir.dt.np(fp8))

    order2 = np.argsort(halfkey, kind="stable")
    s2 = s_sorted[order2]
    slot2 = slot[order2]
    startpos = np.zeros(C * B * 2 + 1, np.int64)
    startpos[1:] = np.cumsum(counts2)

    for c in range(C):
        for b in range(B):
            g = b // Gb
            bi = b - g * Gb
            nb = cfg.nb_of_group[g]
            for hf in range(2):
                k = (c * B + b) * 2 + hf
                cnt = counts2[k]
                call_tile0 = cfg.tile_base[g] + (nb * Th if hf else 0)
                blk_tile0 = call_tile0 + bi * Th
                if cnt > 0:
                    sl = slice(startpos[k], startpos[k + 1])
                    j = np.arange(cnt)
                    gt = blk_tile0 + j // P
                    shalf = s2[sl] - (H if hf else 0)
                    oh[c, j % P,
                       gt * 2 * P + (shalf & 1) * P + slot2[sl]] = 1.0
                    gcol = blk_tile0 * 8 + j // 16
                    idx16[c, j % 16, gcol] = (shalf >> 1).astype(np.int16)

    idx16 = np.tile(idx16, (1, 8, 1))           # replicate 16-row wrap x8

    # per-node-slot pool one-hot (fp8): poh[c, p, b*GOUT + g] = 1
    ids = np.arange(Np)
    bsel = np.where(ids < N, batch[np.minimum(ids, N - 1)], GOUT).astype(np.int64)
    bsel = bsel.reshape(C, B, P)
    poh = np.zeros((C, P, B * GOUT), mybir.dt.np(fp8))
    for c in range(C):
        for b in range(B):
            sel = bsel[c, b]
            valid = sel < GOUT
            pp = np.nonzero(valid)[0]
            poh[c, pp, b * GOUT + sel[pp]] = 1.0

    npad = np.array(
        [max(0, (c + 1) * NshardP - max(N, c * NshardP)) for c in range(C)],
        np.float32,
    )

    # per-node-slot (1 + in_degree), for folding the layer-2 BN affine into
    # layer 3's first matmul: z3 = a2*(v2 + agg(v2)) + (1+deg)*c2
    deg = np.bincount(dst, minlength=Np).astype(np.float32)
    lam = np.ascontiguousarray(
        (1.0 + deg).reshape(C, B, P).transpose(0, 2, 1))    # [C, P, B]

    # per-graph node counts (for the layer-4 pooled-BN fold)
    ng = np.bincount(batch, minlength=GOUT)[:GOUT].astype(np.float32)
    ngrep = np.tile(ng, (P, 1))                                  # [P, GOUT]

    # padded node features (zeros for pad rows) + bf16 gather replica
    x_pad = np.zeros((Np, F), np.float32)
    x_pad[:N] = x
    xb = x_pad.astype(mybir.dt.np(bf16))

    # --- weights ----------------------------------------------------------
    gc_W1 = np.asarray(params["gc_W1"], np.float32)
    gc_W2 = np.asarray(params["gc_W2"], np.float32)
    gc_b1 = np.asarray(params["gc_b1"], np.float32)
    gc_b2 = np.asarray(params["gc_b2"], np.float32)
    gc_g = np.asarray(params["gc_g"], np.float32)
    gc_be = np.asarray(params["gc_be"], np.float32)
    h0_W1 = np.asarray(params["h0_W1"], np.float32)
    h0_W2 = np.asarray(params["h0_W2"], np.float32)
    h1_W1 = np.asarray(params["h1_W1"], np.float32)
    h1_W2 = np.asarray(params["h1_W2"], np.float32)

    W1s = [gc_W1[0], gc_W1[1], gc_W1[2],
           h0_W1.transpose(1, 0, 2).reshape(F, F), _blockdiag(h1_W1)]
    W2s = [gc_W2[0], gc_W2[1], gc_W2[2], _blockdiag(h0_W2), _blockdiag(h1_W2)]
    b1s = [gc_b1[0], gc_b1[1], gc_b1[2],
           np.asarray(params["h0_b1"], np.float32).reshape(-1),
           np.asarray(params["h1_b1"], np.float32).reshape(-1)]
    b2s = [gc_b2[0], gc_b2[1], gc_b2[2],
           np.asarray(params["h0_b2"], np.float32).reshape(-1),
           np.asarray(params["h1_b2"], np.float32).reshape(-1)]
    gs = [gc_g[0], gc_g[1], gc_g[2],
          np.asarray(params["h0_g"], np.float32).reshape(-1),
          np.asarray(params["h1_g"], np.float32).reshape(-1)]
    bes = [gc_be[0], gc_be[1], gc_be[2],
           np.asarray(params["h0_be"], np.float32).reshape(-1),
           np.asarray(params["h1_be"], np.float32).reshape(-1)]

    wpack = np.concatenate(
        sum(([W1s[l], W2s[l]] for l in range(5)), []), axis=1
    ).astype(np.float32)                                        # [F, 1280]
    bpack = np.stack(
        sum(([b1s[l], b2s[l], gs[l], bes[l]] for l in range(5)), []), axis=1
    ).astype(np.float32)                                        # [F, 20]

    in_maps = []
    for c in range(C):
        in_maps.append({
            "xb": xb,
            "xown": np.ascontiguousarray(
                x_pad[c * NshardP:(c + 1) * NshardP]),
            "idx": idx16[c],
            "oh": oh[c],
            "poh": poh[c],
            "lam": lam[c],
            "ngrep": ngrep,
            "npad": np.full((P, 1), npad[c], np.float32),
            "wpack": wpack,
            "bpack": bpack,
        })
    return cfg, in_maps


# ----------------------------------------------------------------------------
# Device program
# ----------------------------------------------------------------------------

def build_device(tc, io, cfg):
    nc = tc.nc
    C, B, Th, Gb = cfg.C, cfg.B, cfg.Th, cfg.Gb
    rg = [list(range(C))]
    AF = mybir.ActivationFunctionType
    OP = mybir.AluOpType

    # internal DRAM ("Shared" outputs only supported for >4-core groups)
    sh = "Shared" if C > 4 else "Local"
    h_dram = nc.dram_tensor("h_rep", [cfg.Np, F], bf16, kind="Internal",
                            addr_space=sh)
    vsh_dram = nc.dram_tensor("v_shard", [cfg.Nshard, F], bf16, kind="Internal")
    st_in = nc.dram_tensor("st_in", [P, 2], f32, kind="Internal")
    st_out = nc.dram_tensor("st_out", [P, 2], f32, kind="Internal",
                            addr_space=sh)
    st4_in = nc.dram_tensor("st4_in", [P, 2 + GOUT], f32, kind="Internal")
    st4_out = nc.dram_tensor("st4_out", [P, 2 + GOUT], f32, kind="Internal",
                             addr_space=sh)

    ctx = ExitStack()
    cpool = ctx.enter_context(tc.tile_pool(name="consts", bufs=1))
    gpool = ctx.enter_context(tc.tile_pool(name="gather", bufs=2))
    opool = ctx.enter_context(tc.tile_pool(name="onehot", bufs=2))
    zpool = ctx.enter_context(tc.tile_pool(name="work", bufs=3))
    spool = ctx.enter_context(tc.tile_pool(name="small", bufs=3))
    ps_agg_pool = ctx.enter_context(tc.tile_pool(name="ps_agg", bufs=2, space="PSUM"))
    ps_mlp_pool = ctx.enter_context(tc.tile_pool(name="ps_mlp", bufs=2, space="PSUM"))
    ps_tr_pool = ctx.enter_context(tc.tile_pool(name="ps_tr", bufs=2, space="PSUM"))
    ps_pool512 = ctx.enter_context(tc.tile_pool(name="ps_p512", bufs=1, space="PSUM"))

    # resident constants / state
    idx_sb = cpool.tile([P, cfg.total_tiles * 8], i16, tag="idx")
    npad_sb = cpool.tile([P, 1], f32, tag="npad")
    w_sb = cpool.tile([P, 10 * F], f32, tag="w")
    bb_sb = cpool.tile([P, 20], f32, tag="bb")
    poh_sb = cpool.tile([P, B * GOUT], fp8, tag="poh")
    lam_sb = cpool.tile([P, B], f32, tag="lam")
    ngrep_sb = cpool.tile([P, GOUT], f32, tag="ngrep")
    w1p3_sb = cpool.tile([P, F], f32, tag="w1p3")
    r3col_sb = cpool.tile([P, 1], f32, tag="r3col")
    r3row_sb = cpool.tile([1, P], f32, tag="r3row")
    vT_sb = cpool.tile([P, B * F], f32, tag="vT")
    hTown_sb = cpool.tile([P, B * F], f32, tag="hTown")
    ssum_sb = cpool.tile([P, B], f32, tag="ssum")
    ssq_sb = cpool.tile([P, B], f32, tag="ssq")
    ident_sb = cpool.tile([P, P], f32, tag="ident")
    hpad_sb = cpool.tile([P, 1], f32, tag="hpad")

    nc.sync.dma_start(out=idx_sb[:], in_=io["idx"][:])
    nc.sync.dma_start(out=npad_sb[:], in_=io["npad"][:])
    nc.sync.dma_start(out=w_sb[:], in_=io["wpack"][:])
    nc.sync.dma_start(out=bb_sb[:], in_=io["bpack"][:])
    nc.sync.dma_start(out=poh_sb[:], in_=io["poh"][:])
    nc.sync.dma_start(out=lam_sb[:], in_=io["lam"][:])
    nc.sync.dma_start(out=ngrep_sb[:], in_=io["ngrep"][:])
    make_identity(nc, ident_sb[:])
    nc.vector.memset(hpad_sb[:], 0.0)
    from concourse import library_config
    nc.gpsimd.load_library(library_config.mlp)

    # hTown <- x_own^T (fp32 self-term, transposed layout)
    for b in range(B):
        xo = zpool.tile([P, F], f32, tag="xo")
        nc.sync.dma_start(out=xo[:], in_=io["xown"][b * P:(b + 1) * P, :])
        ps_x = ps_tr_pool.tile([P, F], f32, tag="tr")
        nc.tensor.transpose(ps_x[:], xo[:], ident_sb[:])
        nc.any.tensor_copy(out=hTown_sb[:, b * F:(b + 1) * F], in_=ps_x[:])

    for l in range(5):
        w1 = w_sb[:, l * 2 * F:(l * 2 + 1) * F]
        w2 = w_sb[:, (l * 2 + 1) * F:(l * 2 + 2) * F]
        b1 = bb_sb[:, 4 * l + 0:4 * l + 1]
        b2 = bb_sb[:, 4 * l + 1:4 * l + 2]
        ga = bb_sb[:, 4 * l + 2:4 * l + 3]
        be = bb_sb[:, 4 * l + 3:4 * l + 4]
        H = cfg.H
        base = io["xb"] if l == 0 else h_dram.ap()
        hv = base.rearrange("(a b) f -> a (b f)", b=2)      # [Np//2, 2F]
        src_lo, src_hi = hv[0:H // 2, :], hv[H // 2:cfg.Np // 2, :]

        # ---- gather + aggregate + MLP, blockwise --------------------------
        if l == 4:
            ps_pool = ps_pool512.tile([P, GOUT], f32, tag="p512")
        for g in range(cfg.n_groups):
            b_lo = g * Gb
            nb = cfg.nb_of_group[g]
            ntc = nb * Th                       # tiles per half-call
            tb0 = cfg.tile_base[g]
            gbuf = gpool.tile([P, 2 * ntc * 2 * F], bf16, tag="gbuf")
            ohbuf = opool.tile([P, 2 * ntc * 2 * P], fp8, tag="ohbuf")
            nc.sync.dma_start(
                out=ohbuf[:],
                in_=io["oh"][:, tb0 * 2 * P:(tb0 + 2 * ntc) * 2 * P])
            ni = ntc * P
            for hf, src_h in ((0, src_lo), (1, src_hi)):
                c0 = (tb0 + hf * ntc) * 8
                nc.gpsimd.dma_gather(
                    gbuf[:, hf * ntc * 2 * F:(hf + 1) * ntc * 2 * F].rearrange(
                        "p (k d) -> p k d", d=2 * F),
                    src_h,
                    idx_sb[:, c0:c0 + ntc * 8],
                    ni, ni, 2 * F, single_packet=False,
                    queue_num=(2 * g + hf) % 4)
            for bi in range(nb):
                b = b_lo + bi
                ps_agg = ps_agg_pool.tile([P, F], f32, tag="agg")
                for t in range(2 * Th):
                    hf, th = (0, t) if t < Th else (1, t - Th)
                    ltile = hf * ntc + bi * Th + th
                    for par in range(2):
                        co = ltile * 2 * P + par * P
                        nc.tensor.matmul(
                            ps_agg[:],
                            lhsT=gbuf[:, co:co + F],
                            rhs=ohbuf[:, co:co + P],
                            start=(t == 0 and par == 0),
                            stop=(t == 2 * Th - 1 and par == 1))
                zT = zpool.tile([P, F], f32, tag="zT")
                self_sb = vT_sb if l == 3 else hTown_sb
                nc.any.tensor_tensor(
                    out=zT[:], in0=ps_agg[:],
                    in1=self_sb[:, b * F:(b + 1) * F], op=OP.add)
                ps_u = ps_mlp_pool.tile([P, F], f32, tag="mlp")
                if l == 3:
                    # layer-2 BN folded in: u = relu(W1'^T z~ + b1 + r3*lam)
                    ps_lt = ps_tr_pool.tile([P, F], f32, tag="tr")
                    nc.tensor.transpose(ps_lt[0:1, :], lam_sb[:, b:b + 1],
                                        ident_sb[:])
                    lamr = spool.tile([1, P], f32, tag="lamr")
                    nc.any.tensor_copy(out=lamr[:], in_=ps_lt[0:1, :])
                    nc.tensor.matmul(ps_u[:], lhsT=w1p3_sb[:], rhs=zT[:],
                                     start=True, stop=False)
                    nc.tensor.matmul(ps_u[:], lhsT=r3row_sb[:], rhs=lamr[:],
                                     start=False, stop=True)
                else:
                    nc.tensor.matmul(ps_u[:], lhsT=w1, rhs=zT[:],
                                     start=True, stop=True)
                uT = zpool.tile([P, F], f32, tag="uT")
                nc.scalar.activation(uT[:], ps_u[:], AF.Relu, bias=b1, scale=1.0)
                ps_v = ps_mlp_pool.tile([P, F], f32, tag="mlp")
                nc.tensor.matmul(ps_v[:], lhsT=w2, rhs=uT[:], start=True, stop=True)
                vT = vT_sb[:, b * F:(b + 1) * F]
                nc.any.tensor_scalar(
                    out=vT, in0=ps_v[:], scalar1=b2, scalar2=None, op0=OP.add,
                    op1=OP.add, accum_out=ssum_sb[:, b:b + 1])
                sq = zpool.tile([P, F], f32, tag="sq")
                nc.scalar.activation(sq[:], vT, AF.Square,
                                     accum_out=ssq_sb[:, b:b + 1])
                if l in (2, 4):
                    # raw (pre-BN) v is what downstream consumes: transpose
                    # now; BN is folded in later (affine, no relu after
                    # layers 2 and 4).
                    ps_t = ps_tr_pool.tile([P, F], f32, tag="tr")
                    nc.tensor.transpose(ps_t[:], vT, ident_sb[:])
                    ntr = zpool.tile([P, F], bf16, tag="ntr")
                    nc.any.tensor_copy(out=ntr[:], in_=ps_t[:])
                    if l == 2:
                        nc.sync.dma_start(
                            out=vsh_dram.ap()[b * P:(b + 1) * P, :], in_=ntr[:])
                    else:
                        nc.tensor.matmul(
                            ps_pool[:], lhsT=ntr[:],
                            rhs=poh_sb[:, b * GOUT:(b + 1) * GOUT],
                            start=(b == 0), stop=(b == B - 1))

        if l == 2:
            # raw v replica can ship before BN stats are even reduced;
            # the stats AllReduce then overlaps layer 3's gathers.
            nc.gpsimd.collective_compute(
                "AllGather", OP.bypass, replica_groups=rg,
                ins=[vsh_dram.ap()], outs=[h_dram.ap()])

        # ---- BN statistics (+ padding-node correction) --------------------
        s1 = spool.tile([P, 1], f32, tag="s1")
        s2 = spool.tile([P, 1], f32, tag="s2")
        nc.vector.reduce_sum(s1[:], ssum_sb[:], axis=mybir.AxisListType.X)
        nc.vector.reduce_sum(s2[:], ssq_sb[:], axis=mybir.AxisListType.X)
        # v_pad = W2.T@relu(W1.T@hpad + b1) + b2  (value of every pad node)
        ps_zp = ps_tr_pool.tile([P, 1], f32, tag="tr")
        if l == 3:
            # hpad holds raw vpad2; apply the folded layer-2 BN (lam=1)
            nc.tensor.matmul(ps_zp[:], lhsT=w1p3_sb[:], rhs=hpad_sb[:],
                             start=True, stop=True)
            zp2 = spool.tile([P, 1], f32, tag="zp2")
            nc.any.tensor_tensor(out=zp2[:], in0=ps_zp[:], in1=r3col_sb[:],
                                 op=OP.add)
            upad_src = zp2
        else:
            nc.tensor.matmul(ps_zp[:], lhsT=w1, rhs=hpad_sb[:],
                             start=True, stop=True)
            upad_src = ps_zp
        upad = spool.tile([P, 1], f32, tag="upad")
        nc.scalar.activation(upad[:], upad_src[:], AF.Relu, bias=b1, scale=1.0)
        ps_vp = ps_tr_pool.tile([P, 1], f32, tag="tr")
        nc.tensor.matmul(ps_vp[:], lhsT=w2, rhs=upad[:], start=True, stop=True)
        vpad = spool.tile([P, 1], f32, tag="vpad")
        nc.any.tensor_scalar(out=vpad[:], in0=ps_vp[:], scalar1=b2,
                             scalar2=None, op0=OP.add)
        t1 = spool.tile([P, 1], f32, tag="t1")
        nc.any.tensor_tensor(out=t1[:], in0=vpad[:], in1=npad_sb[:], op=OP.mult)
        nc.any.tensor_tensor(out=s1[:], in0=s1[:], in1=t1[:], op=OP.subtract)
        vpad2 = spool.tile([P, 1], f32, tag="vpad2")
        nc.scalar.activation(vpad2[:], vpad[:], AF.Square)
        t2 = spool.tile([P, 1], f32, tag="t2")
        nc.any.tensor_tensor(out=t2[:], in0=vpad2[:], in1=npad_sb[:], op=OP.mult)
        nc.any.tensor_tensor(out=s2[:], in0=s2[:], in1=t2[:], op=OP.subtract)

        if l == 4:
            # single AllReduce carries BN stats + raw pooled sums
            stt4 = cpool.tile([P, 2 + GOUT], f32, tag="stt4")
            nc.any.tensor_copy(out=stt4[:, 0:1], in_=s1[:])
            nc.any.tensor_copy(out=stt4[:, 1:2], in_=s2[:])
            nc.any.tensor_copy(out=stt4[:, 2:2 + GOUT], in_=ps_pool[:])
            nc.sync.dma_start(out=st4_in.ap(), in_=stt4[:])
            nc.gpsimd.collective_compute(
                "AllReduce", OP.add, replica_groups=rg,
                ins=[st4_in.ap()], outs=[st4_out.ap()])
            stg4 = cpool.tile([P, 2 + GOUT], f32, tag="stg4")
            nc.sync.dma_start(out=stg4[:], in_=st4_out.ap())
            stg = stg4
        else:
            stt = spool.tile([P, 2], f32, tag="stt")
            nc.any.tensor_copy(out=stt[:, 0:1], in_=s1[:])
            nc.any.tensor_copy(out=stt[:, 1:2], in_=s2[:])
            nc.sync.dma_start(out=st_in.ap(), in_=stt[:])
            nc.gpsimd.collective_compute(
                "AllReduce", OP.add, replica_groups=rg,
                ins=[st_in.ap()], outs=[st_out.ap()])
            stg = spool.tile([P, 2], f32, tag="stg")
            nc.sync.dma_start(out=stg[:], in_=st_out.ap())

        inv_n = 1.0 / float(cfg.N)
        mu = spool.tile([P, 1], f32, tag="mu")
        nc.any.tensor_scalar(out=mu[:], in0=stg[:, 0:1], scalar1=inv_n,
                             scalar2=None, op0=OP.mult)
        ms = spool.tile([P, 1], f32, tag="ms")
        nc.any.tensor_scalar(out=ms[:], in0=stg[:, 1:2], scalar1=inv_n,
                             scalar2=None, op0=OP.mult)
        mu2 = spool.tile([P, 1], f32, tag="mu2")
        nc.scalar.activation(mu2[:], mu[:], AF.Square)
        var = spool.tile([P, 1], f32, tag="var")
        nc.any.tensor_tensor(out=var[:], in0=ms[:], in1=mu2[:], op=OP.subtract)
        veps = spool.tile([P, 1], f32, tag="veps")
        nc.any.tensor_scalar(out=veps[:], in0=var[:], scalar1=BN_EPS,
                             scalar2=None, op0=OP.add)
        sd = spool.tile([P, 1], f32, tag="sd")
        nc.scalar.activation(sd[:], veps[:], AF.Sqrt)
        rs = spool.tile([P, 1], f32, tag="rs")
        nc.vector.reciprocal(rs[:], sd[:])
        aa = spool.tile([P, 1], f32, tag="aa")
        nc.any.tensor_tensor(out=aa[:], in0=rs[:], in1=ga, op=OP.mult)
        mua = spool.tile([P, 1], f32, tag="mua")
        nc.any.tensor_tensor(out=mua[:], in0=mu[:], in1=aa[:], op=OP.mult)
        cc = spool.tile([P, 1], f32, tag="cc")
        nc.any.tensor_tensor(out=cc[:], in0=be, in1=mua[:], op=OP.subtract)

        # hpad' update
        if l == 2:
            nc.any.tensor_copy(out=hpad_sb[:], in_=vpad[:])   # raw: BN folded
        elif l < 4:
            hp1 = spool.tile([P, 1], f32, tag="hp1")
            nc.any.tensor_tensor(out=hp1[:], in0=vpad[:], in1=aa[:], op=OP.mult)
            hp2 = spool.tile([P, 1], f32, tag="hp2")
            nc.any.tensor_tensor(out=hp2[:], in0=hp1[:], in1=cc[:], op=OP.add)
            nc.scalar.activation(hpad_sb[:], hp2[:], AF.Relu)

        if l == 2:
            # prep the layer-3 fold: W1' = diag(a2)@W1_3, r3 = W1_3^T c2
            w1_l3 = w_sb[:, 6 * F:7 * F]
            nc.any.tensor_scalar(out=w1p3_sb[:], in0=w1_l3, scalar1=aa[:],
                                 scalar2=None, op0=OP.mult)
            ps_rc = ps_tr_pool.tile([P, 1], f32, tag="tr")
            nc.tensor.matmul(ps_rc[:], lhsT=w1_l3, rhs=cc[:],
                             start=True, stop=True)
            nc.any.tensor_copy(out=r3col_sb[:], in_=ps_rc[:])
            ps_rr = ps_tr_pool.tile([P, F], f32, tag="tr")
            nc.tensor.matmul(ps_rr[0:1, :], lhsT=cc[:], rhs=w1_l3,
                             start=True, stop=True)
            nc.any.tensor_copy(out=r3row_sb[:], in_=ps_rr[0:1, :])

        # ---- normalize (into hTown) + transpose + AllGather ---------------
        if l in (0, 1, 3):
            for b in range(B):
                nt = hTown_sb[:, b * F:(b + 1) * F]
                nc.scalar.activation(nt, vT_sb[:, b * F:(b + 1) * F],
                                     AF.Relu, bias=cc[:], scale=aa[:])
                ps_t = ps_tr_pool.tile([P, F], f32, tag="tr")
                nc.tensor.transpose(ps_t[:], nt, ident_sb[:])
                ntr = zpool.tile([P, F], bf16, tag="ntr")
                nc.any.tensor_copy(out=ntr[:], in_=ps_t[:])
                nc.sync.dma_start(out=vsh_dram.ap()[b * P:(b + 1) * P, :],
                                  in_=ntr[:])
            nc.gpsimd.collective_compute(
                "AllGather", OP.bypass, replica_groups=rg,
                ins=[vsh_dram.ap()], outs=[h_dram.ap()])

        if l == 4:
            # z_graph = a4 * pooled_raw + n_g * c4, then transpose out
            pg = cpool.tile([P, GOUT], f32, tag="pooledT")
            nc.any.tensor_scalar(out=pg[:], in0=stg4[:, 2:2 + GOUT],
                                 scalar1=aa[:], scalar2=None, op0=OP.mult)
            cg = cpool.tile([P, GOUT], f32, tag="plr")
            nc.any.tensor_scalar(out=cg[:], in0=ngrep_sb[:], scalar1=cc[:],
                                 scalar2=None, op0=OP.mult)
            nc.any.tensor_tensor(out=pg[:], in0=pg[:], in1=cg[:], op=OP.add)
            for q in range(GOUT // P):
                ps_q = ps_tr_pool.tile([P, P], f32, tag="tr")
                nc.tensor.transpose(ps_q[:], pg[:, q * P:(q + 1) * P],
                                    ident_sb[:])
                oq = zpool.tile([P, P], f32, tag="oq")
                nc.any.tensor_copy(out=oq[:], in_=ps_q[:])
                nc.sync.dma_start(out=io["zg"][q * P:(q + 1) * P, :], in_=oq[:])
    ctx.close()


# ----------------------------------------------------------------------------
# Entry point
# ----------------------------------------------------------------------------

_CACHE = {}
_LAST_RESULTS = None


def _build_full(cfg):
    nc = bacc.Bacc("TRN2", target_bir_lowering=False, debug=False,
                   num_devices=cfg.C, num_swdge_queues=4)
    io = {}
    io["xb"] = nc.dram_tensor("xb", [cfg.Np, F], bf16,
                              kind="ExternalInput").ap()
    io["xown"] = nc.dram_tensor("xown", [cfg.Nshard, F], f32,
                                kind="ExternalInput").ap()
    io["idx"] = nc.dram_tensor("idx", [P, cfg.total_tiles * 8], i16,
                               kind="ExternalInput").ap()
    io["oh"] = nc.dram_tensor("oh", [P, cfg.total_tiles * 2 * P], fp8,
                              kind="ExternalInput").ap()
    io["poh"] = nc.dram_tensor("poh", [P, cfg.B * GOUT], fp8,
                               kind="ExternalInput").ap()
    io["lam"] = nc.dram_tensor("lam", [P, cfg.B], f32,
                               kind="ExternalInput").ap()
    io["ngrep"] = nc.dram_tensor("ngrep", [P, GOUT], f32,
                                 kind="ExternalInput").ap()
    io["npad"] = nc.dram_tensor("npad", [P, 1], f32, kind="ExternalInput").ap()
    io["wpack"] = nc.dram_tensor("wpack", [P, 10 * F], f32,
                                 kind="ExternalInput").ap()
    io["bpack"] = nc.dram_tensor("bpack", [P, 20], f32,
                                 kind="ExternalInput").ap()
    io["zg"] = nc.dram_tensor("zg", [GOUT, F], f32, kind="ExternalOutput").ap()
    with tile.TileContext(nc) as tc:
        build_device(tc, io, cfg)
    nc.compile()
    return nc


def kernel(**inputs):
    import os
    global _LAST_RESULTS
    x = np.asarray(inputs["x"], np.float32)
    edge_index = np.asarray(inputs["edge_index"])
    batch = np.asarray(inputs["batch"])
    C = 8
    cfg, in_maps = prep_host(x, edge_index, batch, inputs, C=C)

    key = (x.shape, edge_index.shape, cfg.Th)
    if key not in _CACHE:
        _CACHE[key] = _build_full(cfg)
    nc = _CACHE[key]

    trace = bool(os.environ.get("GNN_TRACE"))
    tmpdir = os.environ.get("GNN_TRACE_DIR") or None
    res = run_bass_kernel_spmd(nc, in_maps, core_ids=list(range(C)),
                               trace=trace, tmpdir=tmpdir)
    _LAST_RESULTS = res
    zg = res.results[0]["zg"]                    # [512, 128]
    return zg.reshape(GOUT, 8, 16).astype(np.float32)
